# revision 4
# baseline (speedup 1.0000x reference)
"""MoE MLP (shared expert weights => plain two-layer GELU MLP) on 8 trn2 cores.

Math (routing is an identity permutation, so gating is dead code):
    h   = gelu(x @ proj1.T + b1)        x: [L, N, E] -> tokens [T=L*N, E]
    out = h @ proj2.T + b2              out: [T, E] -> [L, N, E]

Sharding: data parallel over the token dim (T=16384 -> 2048 tokens/core),
weights replicated. Per core, two chained tile matmuls with the hidden
activation kept in the transposed orientation so no on-chip transpose is
needed:
    pass 1: hT   [H, TS] = gelu(w1T.T @ xT + b1)   (kxm=w1T [E,H], kxn=xT [E,TS])
    pass 2: outT [E, TS] = w2T.T @ hT + b2         (kxm=w2T [H,E], kxn=hT [H,TS])
All transposes (x -> xT per shard, proj -> projT, out.T -> out) happen on the
host in numpy where they are free relative to device time.

DMA-traffic layout choices:
  - pass 1 keeps the full xT shard (16MB) resident in SBUF (kxn_cache), so
    w1 streams exactly once (64MB) and x loads once (16MB).
  - pass 2 contracts K=8192, whose cached operand strips would need 16MB per
    side; instead K is split into two serial 4096-halves whose partial
    products are combined with a DMA accumulate into outT. Each half caches
    its w2T strip (loaded once) and re-streams hT column blocks.

Matmuls run as float32r (fp32 bits, full-rate PE mode) with fp32 PSUM
accumulation; set _USE_F32R = False for the plain (4x slower) fp32 path.
"""

import numpy as np

_L, _N, _E, _H = 2048, 8, 2048, 8192
_T = _L * _N            # 16384 tokens
_NCORES = 8
_TS = _T // _NCORES     # 2048 tokens per core
_P = 128

_USE_F32R = True
_KSPLIT = 2             # serial K-chunks for pass 2

_compiled_nc = None


def _build_nc():
    from contextlib import ExitStack

    import concourse.bacc as bacc
    import concourse.mybir as mybir
    import concourse.tile as tile
    from concourse.kernels.tile_matmul import (
        composable_matmul_tile_kernel,
        dma_from_dram_kxm,
        dma_from_dram_kxn,
        dma_to_dram_mxn,
        k_pool_min_bufs_for_dim,
        scalar_copyback,
    )

    f32 = mybir.dt.float32
    mm_dt = mybir.dt.float32r if _USE_F32R else f32

    nc = bacc.Bacc(None, target_bir_lowering=False, debug=False)
    with tile.TileContext(nc) as tc:
        with ExitStack() as ctx:
            dram = ctx.enter_context(tc.tile_pool(name="dram", bufs=1, space="DRAM"))
            xT = dram.tile([_E, _TS], mm_dt, kind="ExternalInput", name="xT", uniquify=False)
            w1T = dram.tile([_E, _H], mm_dt, kind="ExternalInput", name="w1T", uniquify=False)
            w2T = dram.tile([_H, _E], mm_dt, kind="ExternalInput", name="w2T", uniquify=False)
            b1r = dram.tile([_P, _H // _P], f32, kind="ExternalInput", name="b1r", uniquify=False)
            b2r = dram.tile([_P, _E // _P], f32, kind="ExternalInput", name="b2r", uniquify=False)
            hT = dram.tile([_H, _TS], mm_dt, name="hT", uniquify=False)
            outT = dram.tile([_E, _TS], f32, kind="ExternalOutput", name="outT", uniquify=False)

            const = ctx.enter_context(tc.tile_pool(name="const", bufs=1))
            b1_sb = const.tile([_P, _H // _P], f32, name="b1_sb")
            nc.sync.dma_start(b1_sb[:], b1r[:])
            b2_sb = const.tile([_P, _E // _P], f32, name="b2_sb")
            nc.sync.dma_start(b2_sb[:], b2r[:])

            def gelu_reducer(nc_, psum, sbuf, md):
                # global 128-row group of H for this psum tile
                g = md.m_tile_idx * md.m_subtiles + md.m_subtile_idx
                nc_.scalar.activation(
                    sbuf,
                    psum,
                    mybir.ActivationFunctionType.Gelu,
                    bias=b1_sb[:, g : g + 1],
                )

            def make_bias_reducer(row0):
                g0 = row0 // _P

                def bias_reducer(nc_, psum, sbuf, md):
                    g = g0 + md.m_tile_idx * md.m_subtiles + md.m_subtile_idx
                    nc_.scalar.activation(
                        sbuf,
                        psum,
                        mybir.ActivationFunctionType.Identity,
                        bias=b2_sb[:, g : g + 1],
                    )

                return bias_reducer

            # ---- pass 1: hT = gelu(w1T.T @ xT + b1) ----
            tc.swap_default_side()
            with (
                tc.tile_pool(name="p1_xcache", bufs=1) as xcache_pool,
                tc.tile_pool(
                    name="p1_kxm", bufs=k_pool_min_bufs_for_dim(_E, max_tile_size=256)
                ) as p1_kxm_pool,
                tc.tile_pool(name="p1_kxn_unused", bufs=1) as p1_kxn_pool,
            ):
                xcache = xcache_pool.tile([_P, _E // _P, _TS], mm_dt, name="xcache")
                xT3 = xT[:].rearrange("(po pi) f -> pi po f", pi=_P)
                for i in range(_E // _P):
                    nc.sync.dma_start(xcache[:, i : i + 1, :], xT3[:, i : i + 1, :])

                kxm_producer, kxm_shape = dma_from_dram_kxm(p1_kxm_pool, w1T[:])
                kxn_producer, kxn_shape = dma_from_dram_kxn(
                    p1_kxn_pool, xT[:], kxn_cache=xcache[:]
                )
                composable_matmul_tile_kernel(
                    tc,
                    kxm_shape=kxm_shape,
                    kxn_shape=kxn_shape,
                    output_type=mm_dt,
                    kxm_producer=kxm_producer,
                    kxn_producer=kxn_producer,
                    mxn_consumer=dma_to_dram_mxn(hT[:]),
                    mxn_subtile_reducer=gelu_reducer,
                    MAX_K_TILE_SIZE=256,
                    temps_n_bufs=2,
                    psum_n_bufs=2,
                )

            # ---- pass 2: outT = w2T.T @ hT + b2, K split into serial chunks ----
            kc = _H // _KSPLIT
            for s in range(_KSPLIT):
                tc.swap_default_side()
                nbufs = k_pool_min_bufs_for_dim(kc, max_tile_size=512)
                with (
                    tc.tile_pool(name=f"p2_kxm_{s}", bufs=nbufs) as p2_kxm_pool,
                    tc.tile_pool(name=f"p2_kxn_{s}", bufs=nbufs) as p2_kxn_pool,
                ):
                    kxm_producer, kxm_shape = dma_from_dram_kxm(
                        p2_kxm_pool, w2T[s * kc : (s + 1) * kc, :]
                    )
                    kxn_producer, kxn_shape = dma_from_dram_kxn(
                        p2_kxn_pool, hT[s * kc : (s + 1) * kc, :]
                    )
                    last = s == _KSPLIT - 1
                    composable_matmul_tile_kernel(
                        tc,
                        kxm_shape=kxm_shape,
                        kxn_shape=kxn_shape,
                        output_type=f32,
                        kxm_producer=kxm_producer,
                        kxn_producer=kxn_producer,
                        mxn_consumer=dma_to_dram_mxn(
                            outT[:],
                            accum_op=(
                                mybir.AluOpType.add if s > 0 else mybir.AluOpType.bypass
                            ),
                        ),
                        mxn_subtile_reducer=(
                            make_bias_reducer(0) if last else scalar_copyback()
                        ),
                        MAX_K_TILE_SIZE=512,
                        temps_n_bufs=2,
                        psum_n_bufs=2,
                    )

    nc.compile()
    return nc


def _get_nc():
    global _compiled_nc
    if _compiled_nc is None:
        _compiled_nc = _build_nc()
    return _compiled_nc


def _make_in_maps(x, proj1, proj1_bias, proj2, proj2_bias):
    xt = np.ascontiguousarray(x.reshape(_T, _E))
    w1T = np.ascontiguousarray(proj1.T)  # [E, H]
    w2T = np.ascontiguousarray(proj2.T)  # [H, E]
    b1r = np.ascontiguousarray(proj1_bias.reshape(_H // _P, _P).T)  # [128, H/128]
    b2r = np.ascontiguousarray(proj2_bias.reshape(_E // _P, _P).T)  # [128, E/128]
    in_maps = []
    for c in range(_NCORES):
        shard = xt[c * _TS : (c + 1) * _TS]  # [TS, E]
        in_maps.append(
            {
                "xT": np.ascontiguousarray(shard.T),  # [E, TS]
                "w1T": w1T,
                "w2T": w2T,
                "b1r": b1r,
                "b2r": b2r,
            }
        )
    return in_maps


def kernel(x, proj1, proj1_bias, proj2, proj2_bias, gate_w=None, **_ignored):
    # gate_w only affects the (dead) routing ids, never the output.
    from concourse.bass_utils import run_bass_kernel_spmd

    nc = _get_nc()
    in_maps = _make_in_maps(
        np.asarray(x, np.float32),
        np.asarray(proj1, np.float32),
        np.asarray(proj1_bias, np.float32),
        np.asarray(proj2, np.float32),
        np.asarray(proj2_bias, np.float32),
    )
    res = run_bass_kernel_spmd(nc, in_maps, list(range(_NCORES)))
    out = np.empty((_T, _E), np.float32)
    for c in range(_NCORES):
        out[c * _TS : (c + 1) * _TS] = res.results[c]["outT"].T
    return out.reshape(_L, _N, _E)


# revision 6
# speedup vs baseline: 1.2057x; 1.2057x over previous
"""MoE MLP (shared expert weights => plain two-layer GELU MLP) on 8 trn2 cores.

Math (routing is an identity permutation, so gating is dead code):
    h   = gelu(x @ proj1.T + b1)        x: [L, N, E] -> tokens [T=L*N, E]
    out = h @ proj2.T + b2              out: [T, E] -> [L, N, E]

Sharding: data parallel over the token dim (T=16384 -> 2048 tokens/core),
weights replicated. Per core, two chained tile matmuls with the hidden
activation kept in the transposed orientation so no on-chip transpose is
needed:
    pass 1: hT   [H, TS] = gelu(w1T.T @ xT + b1)   (kxm=w1T [E,H], kxn=xT [E,TS])
    pass 2: outT [E, TS] = w2T.T @ hT + b2         (kxm=w2T [H,E], kxn=hT [H,TS])
All transposes (x -> xT per shard, proj -> projT, out.T -> out) happen on the
host in numpy where they are free relative to device time.

DMA-traffic layout choices:
  - pass 1 keeps the full xT shard (16MB) resident in SBUF (kxn_cache), so
    w1 streams exactly once (64MB) and x loads once (16MB).
  - pass 2 contracts K=8192, whose cached operand strips would need 16MB per
    side; instead K is split into two serial 4096-halves whose partial
    products are combined with a DMA accumulate into outT. Each half caches
    its w2T strip (loaded once) and re-streams hT column blocks.

Matmuls run as float32r (fp32 bits, full-rate PE mode) with fp32 PSUM
accumulation; set _USE_F32R = False for the plain (4x slower) fp32 path.
"""

import numpy as np

_L, _N, _E, _H = 2048, 8, 2048, 8192
_T = _L * _N            # 16384 tokens
_NCORES = 8
_TS = _T // _NCORES     # 2048 tokens per core
_P = 128

_USE_F32R = True
_KSPLIT = 2             # serial K-chunks for pass 2

_compiled_nc = None


def _build_nc():
    from contextlib import ExitStack

    import concourse.bacc as bacc
    import concourse.mybir as mybir
    import concourse.tile as tile
    from concourse.bass import ts as bass_ts
    from concourse.kernels.tile_matmul import (
        composable_matmul_tile_kernel,
        dma_from_dram_kxm,
        dma_from_dram_kxn,
        dma_to_dram_mxn,
        k_pool_min_bufs_for_dim,
        scalar_copyback,
    )

    f32 = mybir.dt.float32
    mm_dt = mybir.dt.float32r if _USE_F32R else f32

    nc = bacc.Bacc(None, target_bir_lowering=False, debug=False)
    with tile.TileContext(nc) as tc:
        with ExitStack() as ctx:
            dram = ctx.enter_context(tc.tile_pool(name="dram", bufs=1, space="DRAM"))
            xT = dram.tile([_E, _TS], mm_dt, kind="ExternalInput", name="xT", uniquify=False)
            w1T = dram.tile([_E, _H], mm_dt, kind="ExternalInput", name="w1T", uniquify=False)
            w2T = dram.tile([_H, _E], mm_dt, kind="ExternalInput", name="w2T", uniquify=False)
            b1r = dram.tile([_P, _H // _P], f32, kind="ExternalInput", name="b1r", uniquify=False)
            b2r = dram.tile([_P, _E // _P], f32, kind="ExternalInput", name="b2r", uniquify=False)
            hT = dram.tile([_H, _TS], mm_dt, name="hT", uniquify=False)
            outT = dram.tile([_E, _TS], f32, kind="ExternalOutput", name="outT", uniquify=False)

            const = ctx.enter_context(tc.tile_pool(name="const", bufs=1))
            b1_sb = const.tile([_P, _H // _P], f32, name="b1_sb")
            nc.sync.dma_start(b1_sb[:], b1r[:])
            b2_sb = const.tile([_P, _E // _P], f32, name="b2_sb")
            nc.sync.dma_start(b2_sb[:], b2r[:])

            def gelu_reducer(nc_, psum, sbuf, md):
                # global 128-row group of H for this psum tile
                g = md.m_tile_idx * md.m_subtiles + md.m_subtile_idx
                nc_.scalar.activation(
                    sbuf,
                    psum,
                    mybir.ActivationFunctionType.Gelu,
                    bias=b1_sb[:, g : g + 1],
                )

            def make_bias_reducer(row0):
                g0 = row0 // _P

                def bias_reducer(nc_, psum, sbuf, md):
                    g = g0 + md.m_tile_idx * md.m_subtiles + md.m_subtile_idx
                    nc_.scalar.activation(
                        sbuf,
                        psum,
                        mybir.ActivationFunctionType.Identity,
                        bias=b2_sb[:, g : g + 1],
                    )

                return bias_reducer

            # ---- pass 1: hT = gelu(w1T.T @ xT + b1) ----
            tc.swap_default_side()
            with (
                tc.tile_pool(name="p1_xcache", bufs=1) as xcache_pool,
                tc.tile_pool(
                    name="p1_kxm", bufs=k_pool_min_bufs_for_dim(_E, max_tile_size=256)
                ) as p1_kxm_pool,
                tc.tile_pool(name="p1_kxn_unused", bufs=1) as p1_kxn_pool,
            ):
                xcache = xcache_pool.tile([_P, _E // _P, _TS], mm_dt, name="xcache")
                xT3 = xT[:].rearrange("(po pi) f -> pi po f", pi=_P)
                for i in range(_E // _P):
                    nc.sync.dma_start(xcache[:, i : i + 1, :], xT3[:, i : i + 1, :])

                kxm_producer, kxm_shape = dma_from_dram_kxm(p1_kxm_pool, w1T[:])
                # zero-DMA producer: x stays resident in SBUF for all of pass 1
                _, kxn_shape = dma_from_dram_kxn(p1_kxn_pool, xT[:])

                def kxn_producer(nc_, md):
                    return xcache[
                        :,
                        bass_ts(md.k_tile_idx, md.k_subtiles),
                        bass_ts(md.n_tile_idx, md.n_tile),
                    ]
                composable_matmul_tile_kernel(
                    tc,
                    kxm_shape=kxm_shape,
                    kxn_shape=kxn_shape,
                    output_type=mm_dt,
                    kxm_producer=kxm_producer,
                    kxn_producer=kxn_producer,
                    mxn_consumer=dma_to_dram_mxn(hT[:]),
                    mxn_subtile_reducer=gelu_reducer,
                    MAX_K_TILE_SIZE=256,
                    temps_n_bufs=2,
                    psum_n_bufs=2,
                )

            # ---- pass 2: outT = w2T.T @ hT + b2, K split into serial chunks ----
            kc = _H // _KSPLIT
            for s in range(_KSPLIT):
                tc.swap_default_side()
                nbufs = k_pool_min_bufs_for_dim(kc, max_tile_size=512)
                with (
                    tc.tile_pool(name=f"p2_kxm_{s}", bufs=nbufs) as p2_kxm_pool,
                    tc.tile_pool(name=f"p2_kxn_{s}", bufs=nbufs) as p2_kxn_pool,
                ):
                    kxm_producer, kxm_shape = dma_from_dram_kxm(
                        p2_kxm_pool, w2T[s * kc : (s + 1) * kc, :]
                    )
                    kxn_producer, kxn_shape = dma_from_dram_kxn(
                        p2_kxn_pool, hT[s * kc : (s + 1) * kc, :]
                    )
                    last = s == _KSPLIT - 1
                    composable_matmul_tile_kernel(
                        tc,
                        kxm_shape=kxm_shape,
                        kxn_shape=kxn_shape,
                        output_type=f32,
                        kxm_producer=kxm_producer,
                        kxn_producer=kxn_producer,
                        mxn_consumer=dma_to_dram_mxn(
                            outT[:],
                            accum_op=(
                                mybir.AluOpType.add if s > 0 else mybir.AluOpType.bypass
                            ),
                        ),
                        mxn_subtile_reducer=(
                            make_bias_reducer(0) if last else scalar_copyback()
                        ),
                        MAX_K_TILE_SIZE=512,
                        temps_n_bufs=2,
                        psum_n_bufs=2,
                    )

    nc.compile()
    return nc


def _get_nc():
    global _compiled_nc
    if _compiled_nc is None:
        _compiled_nc = _build_nc()
    return _compiled_nc


def _make_in_maps(x, proj1, proj1_bias, proj2, proj2_bias):
    xt = np.ascontiguousarray(x.reshape(_T, _E))
    w1T = np.ascontiguousarray(proj1.T)  # [E, H]
    w2T = np.ascontiguousarray(proj2.T)  # [H, E]
    b1r = np.ascontiguousarray(proj1_bias.reshape(_H // _P, _P).T)  # [128, H/128]
    b2r = np.ascontiguousarray(proj2_bias.reshape(_E // _P, _P).T)  # [128, E/128]
    in_maps = []
    for c in range(_NCORES):
        shard = xt[c * _TS : (c + 1) * _TS]  # [TS, E]
        in_maps.append(
            {
                "xT": np.ascontiguousarray(shard.T),  # [E, TS]
                "w1T": w1T,
                "w2T": w2T,
                "b1r": b1r,
                "b2r": b2r,
            }
        )
    return in_maps


def kernel(x, proj1, proj1_bias, proj2, proj2_bias, gate_w=None, **_ignored):
    # gate_w only affects the (dead) routing ids, never the output.
    from concourse.bass_utils import run_bass_kernel_spmd

    nc = _get_nc()
    in_maps = _make_in_maps(
        np.asarray(x, np.float32),
        np.asarray(proj1, np.float32),
        np.asarray(proj1_bias, np.float32),
        np.asarray(proj2, np.float32),
        np.asarray(proj2_bias, np.float32),
    )
    res = run_bass_kernel_spmd(nc, in_maps, list(range(_NCORES)))
    out = np.empty((_T, _E), np.float32)
    for c in range(_NCORES):
        out[c * _TS : (c + 1) * _TS] = res.results[c]["outT"].T
    return out.reshape(_L, _N, _E)
